# revision 1
# baseline (speedup 1.0000x reference)
"""Trainium2 Bass kernel for CausalSE (chunked-mean-pool -> per-channel EMA ->
int4-fake-quant SE bottleneck -> sigmoid gate -> gated residual).

Contract: kernel(**inputs) takes FULL unsharded inputs (as produced by
setup_inputs) and returns the FULL [16, 2048, 4096] float32 output.
Internally shards batch 16 -> 8 NeuronCores (2 per core), replicating the
small weights, and runs a single-pass streaming Bass/Tile kernel per core.

Algorithm notes:
  - pooled mean and the (1-r) EMA input scale are folded into the first SE
    matmul weights: scan computes q[t] = r*q[t-1] + chunk_sum[t], and
    W1' = fq(w1) * ((1-r)/16) per input channel, so h = s*q never needs to be
    materialized.
  - EMA runs as one hardware TensorTensorScan per (batch, time-block) over the
    flattened (channel-block, pooled-t) axis; r is masked to 0 at each
    channel-block's first pooled step so segments don't leak, and the carry
    from the previous time block is injected into the first chunk-sum.
  - Weight fake-quant (int4 symmetric, round-half-even) is exact host-side
    preprocessing of tiny tensors; all x-dependent compute runs on device.

Performance notes (the kernel is HBM-DMA-bound; each choice below keeps the
other engines hidden underneath the 64 MiB/core of streaming traffic):
  - x and the output stream as bf16 (host casts): halves HBM bytes for
    ~3e-3 relative error, well inside the 2e-2 budget. SE matmul weights and
    activations are bf16 too (HW fp32 matmul is ~4x slower).
  - the host pre-permutes x to [batch, block, partition, chan-blk, t] so each
    512-step block is one fully contiguous 2 MiB DMA per direction.
  - chunk-pooling runs as a pairwise bf16 tensor_add tree (DVE tensor_reduce
    only has a 1x uop; all-step-1 bf16 tensor_tensor gets the 2x packed mode).
  - the sigmoid reads the whole block's PSUM bank through a stride-0 AP and
    writes the gate already expanded to chunk resolution, so the big gating
    multiply is step-1 on both operands (2x mode); the per-channel-block bias
    is applied by a rank-1 matmul on the otherwise idle PE.
  - xbufs=3 measured faster than deeper buffering (6-8) on hardware, and
    interleaving the two batch elements' blocks (b0k0, b1k0, b0k1, ...)
    doubles the slack on the EMA carry chain, hiding the SE-chain latency.
"""

import contextlib

import ml_dtypes
import numpy as np

import concourse.bacc as bacc
import concourse.mybir as mybir
import concourse.tile as tile
from concourse import bass_utils

F32 = mybir.dt.float32
BF16 = mybir.dt.bfloat16
NP_BF16 = ml_dtypes.bfloat16

B = 16
C = 2048
L = 4096
CHUNK = 16
HID = 256
QMAX = 7
EPS = 1e-5
N_CORES = 8
BPC = B // N_CORES          # batches per core = 2
P = 128
NCB = C // P                # channel blocks = 16
NOC = HID // P              # hidden (SE bottleneck) blocks = 2
TBLK = 512                  # time elements per streamed block
NTB = L // TBLK             # time blocks = 8
TP = TBLK // CHUNK          # pooled steps per block = 32
CONTIG = 1                  # host pre-permutes x so block DMAs are contiguous
GEXP = 1                    # expand gate to chunk resolution on ACT

_CACHE = {}


def _emit_body(nc, xd, outd, w1, w2, b1, b2, rmask, rlast,
               xpool, spool, carrypool, ps1, ps2, tblk=TBLK, ablate=(),
               contig=0, rdeng=0, muleng=0, treered=0, gexp=0,
               ones=None, b2row=None, interleave=0, fo=0):
    """One full pass over this core's two batch elements.

    Emission is software-pipelined: each (b, k) iteration emits this block's
    load/pool/scan/SE-gate, but the gate-multiply + store of the PREVIOUS
    block. Engine instruction streams execute in order, so emitting mul(k)
    right after gate(k) would stall the whole DVE stream on the PE/ACT SE
    chain; delaying it one block keeps DVE busy with pooling while the tiny
    SE matmuls for the previous block finish on PE/ACT.
    """
    ntb = L // tblk
    tp = tblk // CHUNK

    pending = None  # (xt, gate, b, t0) awaiting mul+store

    def flush_pending():
        nonlocal pending
        if pending is None:
            return
        xt, gate, b, k = pending
        if "mul" not in ablate:
            x4 = xt[:].rearrange("p cb (tp ch) -> p cb tp ch", ch=CHUNK)
            if gexp:
                # gate already expanded to chunk resolution: both operands
                # step-1 bf16 -> DVE 2x packed mode
                nc.vector.tensor_mul(x4, x4, gate[:])
            else:
                gb = gate[:].unsqueeze(3).broadcast_to([P, NCB, tp, CHUNK])
                if muleng == 0:
                    nc.vector.tensor_mul(x4, x4, gb)
                elif muleng == 1:
                    nc.gpsimd.tensor_mul(x4, x4, gb)
                else:
                    h = NCB // 2
                    nc.vector.tensor_mul(x4[:, :h], x4[:, :h], gb[:, :h])
                    nc.gpsimd.tensor_mul(x4[:, h:], x4[:, h:], gb[:, h:])
        if contig:
            nc.scalar.dma_start(outd.ap()[b][k], xt[:])
        else:
            t0 = k * tblk
            nc.scalar.dma_start(
                outd.ap()[b][:, :, t0:t0 + tblk].transpose([1, 0, 2]),
                xt[:],
            )
        pending = None

    if interleave:
        sched = [(b, k) for k in range(ntb) for b in range(BPC)]
    else:
        sched = [(b, k) for b in range(BPC) for k in range(ntb)]
    qcs = []
    for b in range(BPC):
        qc_t = carrypool.tile([P, NCB], F32, tag=f"qc{b}")
        qcs.append(qc_t)
    if True:
        for b, k in sched:
            qc = qcs[b]
            xt = xpool.tile([P, NCB, tblk], BF16, tag="xt")
            if contig:
                nc.sync.dma_start(xt[:], xd.ap()[b][k])
            else:
                t0 = k * tblk
                nc.sync.dma_start(
                    xt[:],
                    xd.ap()[b][:, :, t0:t0 + tblk].transpose([1, 0, 2]),
                )
            x4 = xt[:].rearrange("p cb (tp ch) -> p cb tp ch", ch=CHUNK)

            sums = spool.tile([P, NCB, tp], F32, tag="sums")
            if "reduce" in ablate:
                nc.gpsimd.memset(sums[:], 0.01)
            elif treered:
                # pairwise-add tree: every level is all-bf16 step-1, so DVE
                # runs it in the 2x packed mode (tensor_reduce only has a 1x
                # uop and would cost ~2x more)
                tr = spool.tile([P, NCB, tp, 8], BF16, tag="tr")
                nc.vector.tensor_add(tr[:], x4[:, :, :, 0:8], x4[:, :, :, 8:16])
                nc.vector.tensor_add(tr[:, :, :, 0:4], tr[:, :, :, 0:4],
                                     tr[:, :, :, 4:8])
                nc.vector.tensor_add(tr[:, :, :, 0:2], tr[:, :, :, 0:2],
                                     tr[:, :, :, 2:4])
                nc.vector.tensor_add(sums[:], tr[:, :, :, 0], tr[:, :, :, 1])
            else:
                reng = nc.gpsimd if rdeng else nc.vector
                reng.reduce_sum(sums[:], x4, axis=mybir.AxisListType.X)

            if "se" in ablate:
                flush_pending()
                pending = (xt, sums, b, k)
                continue
            if k > 0:
                tmp = spool.tile([P, NCB], F32, tag="tmp")
                nc.vector.tensor_mul(tmp[:], qc[:], rlast[:])
                nc.vector.tensor_add(sums[:, :, 0], sums[:, :, 0], tmp[:])

            q = spool.tile([P, NCB, tp], BF16, tag="q")
            nc.vector.tensor_tensor_scan(
                q[:].rearrange("p cb tp -> p (cb tp)"),
                rmask[:].rearrange("p cb tp -> p (cb tp)"),
                sums[:].rearrange("p cb tp -> p (cb tp)"),
                initial=0.0,
                op0=mybir.AluOpType.mult,
                op1=mybir.AluOpType.add,
            )
            if k < ntb - 1:
                nc.vector.tensor_copy(qc[:], q[:, :, tp - 1])

            if not fo:
                flush_pending()

            h1 = spool.tile([P, NOC, tp], BF16, tag="h1")
            for oc in range(NOC):
                acc = ps1.tile([P, tp], F32, tag="acc1")
                for cb in range(NCB):
                    nc.tensor.matmul(
                        acc[:],
                        w1[:, cb, oc * P:(oc + 1) * P],
                        q[:, cb, :],
                        start=(cb == 0),
                        stop=(cb == NCB - 1),
                    )
                nc.scalar.activation(
                    h1[:, oc, :], acc[:],
                    mybir.ActivationFunctionType.Relu,
                    bias=b1[:, oc:oc + 1],
                )

            if gexp:
                # all output blocks accumulate into one PSUM bank; per-block
                # bias lands via a 1-partition rank-1 matmul so a single
                # sigmoid (split in two for the PSUM 4K free-dim cap) can
                # write the gate already chunk-expanded for a 2x-mode mul
                acc2 = ps2.tile([P, NCB, tp], F32, tag="acc2big")
                for ob in range(NCB):
                    for kc in range(NOC):
                        nc.tensor.matmul(
                            acc2[:, ob, :],
                            w2[:, kc, ob * P:(ob + 1) * P],
                            h1[:, kc, :],
                            start=(kc == 0),
                            stop=False,
                        )
                    nc.tensor.matmul(
                        acc2[:, ob, :],
                        b2row[0:1, ob * P:(ob + 1) * P],
                        ones[0:1, :tp],
                        start=False,
                        stop=True,
                    )
                gate = spool.tile([P, NCB, tp, CHUNK], BF16, tag="gate16")
                if gexp == 2:
                    # sigmoid writes adjacent bf16 pairs; one int32-view copy
                    # replicates pairs to chunk width (half the elements)
                    g2 = spool.tile([P, NCB, tp, 2], BF16, tag="g2")
                    nc.scalar.activation(
                        g2[:], acc2[:].unsqueeze(3).broadcast_to(
                            [P, NCB, tp, 2]),
                        mybir.ActivationFunctionType.Sigmoid)
                    u32 = mybir.dt.uint32
                    nc.vector.tensor_copy(
                        gate[:].bitcast(u32),
                        g2[:].bitcast(u32).broadcast_to(
                            [P, NCB, tp, CHUNK // 2]),
                    )
                else:
                    gb = acc2[:].unsqueeze(3).broadcast_to(
                        [P, NCB, tp, CHUNK])
                    half = NCB // 2
                    nc.scalar.activation(
                        gate[:, :half], gb[:, :half],
                        mybir.ActivationFunctionType.Sigmoid)
                    nc.scalar.activation(
                        gate[:, half:], gb[:, half:],
                        mybir.ActivationFunctionType.Sigmoid)
            else:
                gate = spool.tile([P, NCB, tp], BF16, tag="gate")
                for ob in range(NCB):
                    acc2 = ps2.tile([P, tp], F32, tag="acc2")
                    for kc in range(NOC):
                        nc.tensor.matmul(
                            acc2[:],
                            w2[:, kc, ob * P:(ob + 1) * P],
                            h1[:, kc, :],
                            start=(kc == 0),
                            stop=(kc == NOC - 1),
                        )
                    nc.scalar.activation(
                        gate[:, ob, :], acc2[:],
                        mybir.ActivationFunctionType.Sigmoid,
                        bias=b2[:, ob:ob + 1],
                    )

            if fo:
                flush_pending()
            pending = (xt, gate, b, k)
    flush_pending()


def _build_module(repeat=1, tblk=TBLK, xbufs=3, sbufs=2, ps1b=2, ps2b=4, ablate=(),
                  contig=CONTIG, rdeng=0, muleng=0, treered=1, gexp=GEXP,
                  interleave=1, fo=0):
    """Build the per-core module. repeat>1 wraps the body in a hardware loop
    that re-runs it (idempotently) for slope-based device timing."""
    tp = tblk // CHUNK
    ntb = L // tblk
    nc = bacc.Bacc("TRN2", target_bir_lowering=False, debug=False,
                   num_devices=N_CORES)

    xshape = [BPC, ntb, P, NCB, tblk] if contig else [BPC, NCB, P, L]
    xd = nc.dram_tensor("x", xshape, BF16, kind="ExternalInput")
    w1d = nc.dram_tensor("w1t", [P, NCB, HID], BF16, kind="ExternalInput")
    w2d = nc.dram_tensor("w2t", [P, NOC, C], BF16, kind="ExternalInput")
    b1d = nc.dram_tensor("b1t", [P, NOC], F32, kind="ExternalInput")
    b2d = nc.dram_tensor("b2t", [P, NCB], F32, kind="ExternalInput")
    rmd = nc.dram_tensor("rmask", [P, NCB, tp], F32, kind="ExternalInput")
    rld = nc.dram_tensor("rlast", [P, NCB], F32, kind="ExternalInput")
    b2rd = (nc.dram_tensor("b2r", [1, C], BF16, kind="ExternalInput")
            if gexp else None)
    outd = nc.dram_tensor("out", xshape, BF16, kind="ExternalOutput")

    with tile.TileContext(nc) as tc:
        with (
            tc.tile_pool(name="const", bufs=1) as cpool,
            tc.tile_pool(name="xp", bufs=xbufs) as xpool,
            tc.tile_pool(name="small", bufs=sbufs) as spool,
            tc.tile_pool(name="carry", bufs=1) as carrypool,
            tc.tile_pool(name="ps1", bufs=ps1b, space="PSUM") as ps1,
            tc.tile_pool(name="ps2", bufs=ps2b, space="PSUM") as ps2,
        ):
            w1 = cpool.tile([P, NCB, HID], BF16)
            w2 = cpool.tile([P, NOC, C], BF16)
            b1 = cpool.tile([P, NOC], F32)
            b2 = cpool.tile([P, NCB], F32)
            rmask = cpool.tile([P, NCB, tp], F32)
            rlast = cpool.tile([P, NCB], F32)
            nc.gpsimd.dma_start(w1[:], w1d.ap())
            nc.gpsimd.dma_start(w2[:], w2d.ap())
            nc.gpsimd.dma_start(b1[:], b1d.ap())
            nc.gpsimd.dma_start(b2[:], b2d.ap())
            nc.gpsimd.dma_start(rmask[:], rmd.ap())
            nc.gpsimd.dma_start(rlast[:], rld.ap())
            if gexp:
                b2row = cpool.tile([1, C], BF16)
                nc.gpsimd.dma_start(b2row[:], b2rd.ap())
                ones = cpool.tile([1, tp], BF16)
                nc.gpsimd.memset(ones[:], 1.0)
            else:
                b2row = ones = None

            rep = tc.For_i(0, repeat, 1) if repeat > 1 else contextlib.nullcontext()
            with rep:
                _emit_body(nc, xd, outd, w1, w2, b1, b2, rmask, rlast,
                           xpool, spool, carrypool, ps1, ps2, tblk=tblk,
                           ablate=ablate, contig=contig, rdeng=rdeng,
                           muleng=muleng, treered=treered, gexp=gexp,
                           ones=ones, b2row=b2row, interleave=interleave,
                           fo=fo)

    nc.compile()
    return nc


def _fake_quant(w):
    w = np.asarray(w, np.float32)
    scale = (np.max(np.abs(w), axis=1, keepdims=True).astype(np.float32)
             / np.float32(QMAX) + np.float32(EPS)).astype(np.float32)
    wq = np.clip(np.round(w / scale), -QMAX, QMAX).astype(np.float32) * scale
    return wq.astype(np.float32)


def _host_prep(w1, b1, w2, b2, ema_r, tp=TP):
    r = np.asarray(ema_r, np.float32)
    s = ((np.float32(1.0) - r) / np.float32(CHUNK)).astype(np.float32)

    w1s = (_fake_quant(w1) * s[None, :]).astype(np.float32)        # [HID, C]
    w1t = np.ascontiguousarray(
        w1s.T.reshape(NCB, P, HID).transpose(1, 0, 2)).astype(NP_BF16)
    w2q = _fake_quant(w2)                                          # [C, HID]
    w2t = np.ascontiguousarray(
        w2q.T.reshape(NOC, P, C).transpose(1, 0, 2)).astype(NP_BF16)
    b1t = np.ascontiguousarray(np.asarray(b1, np.float32).reshape(NOC, P).T)
    b2t = np.ascontiguousarray(np.asarray(b2, np.float32).reshape(NCB, P).T)

    rpb = r.reshape(NCB, P).T                                      # [P, NCB]
    rmask = np.repeat(rpb[:, :, None], tp, axis=2).astype(np.float32)
    rmask[:, :, 0] = 0.0
    rlast = np.ascontiguousarray(rpb)
    return w1t, w2t, b1t, b2t, np.ascontiguousarray(rmask), rlast


def _make_in_maps(x, w1, b1, w2, b2, ema_r, tp=TP):
    w1t, w2t, b1t, b2t, rmask, rlast = _host_prep(w1, b1, w2, b2, ema_r, tp=tp)
    xh = np.asarray(x, np.float32).reshape(B, NCB, P, L).astype(NP_BF16)
    if CONTIG:
        tblk = tp * CHUNK
        xh = np.ascontiguousarray(
            xh.reshape(B, NCB, P, L // tblk, tblk).transpose(0, 3, 2, 1, 4))
    m = {"w1t": w1t, "w2t": w2t, "b1t": b1t, "b2t": b2t,
         "rmask": rmask, "rlast": rlast}
    if GEXP:
        m["b2r"] = np.asarray(b2, np.float32).reshape(1, C).astype(NP_BF16)
    return [dict(m, x=xh[c * BPC:(c + 1) * BPC]) for c in range(N_CORES)]


def kernel(x, w1, b1, w2, b2, ema_r):
    if "nc" not in _CACHE:
        _CACHE["nc"] = _build_module()
    nc = _CACHE["nc"]

    in_maps = _make_in_maps(x, w1, b1, w2, b2, ema_r)
    res = bass_utils.run_bass_kernel_spmd(nc, in_maps,
                                          core_ids=list(range(N_CORES)))
    if CONTIG:
        out = np.empty((B, NTB, P, NCB, TBLK), np.float32)
        for c in range(N_CORES):
            out[c * BPC:(c + 1) * BPC] = np.asarray(res.results[c]["out"])
        out = np.ascontiguousarray(out.transpose(0, 3, 2, 1, 4))
    else:
        out = np.empty((B, NCB, P, L), np.float32)
        for c in range(N_CORES):
            out[c * BPC:(c + 1) * BPC] = np.asarray(res.results[c]["out"])
    return out.reshape(B, C, L)



# revision 50
# speedup vs baseline: 1.2750x; 1.2750x over previous
"""Trainium2 Bass kernel for CausalSE (chunked-mean-pool -> per-channel EMA ->
int4-fake-quant SE bottleneck -> sigmoid gate -> gated residual).

Contract: kernel(**inputs) takes FULL unsharded inputs (as produced by
setup_inputs) and returns the FULL [16, 2048, 4096] float32 output.
Internally shards batch 16 -> 8 NeuronCores (2 per core), replicating the
small weights, and runs a single-pass streaming Bass/Tile kernel per core.

Algorithm notes:
  - pooled mean and the (1-r) EMA input scale are folded into the first SE
    matmul weights: scan computes q[t] = r*q[t-1] + chunk_sum[t], and
    W1' = fq(w1) * ((1-r)/16) per input channel, so h = s*q never needs to be
    materialized.
  - EMA runs as one hardware TensorTensorScan per (batch, time-block) over the
    flattened (channel-block, pooled-t) axis; r is masked to 0 at each
    channel-block's first pooled step so segments don't leak, and the carry
    from the previous time block is injected into the first chunk-sum.
  - Weight fake-quant (int4 symmetric, round-half-even) is exact host-side
    preprocessing of tiny tensors; all x-dependent compute runs on device.

Performance notes (the kernel is HBM-DMA-bound; each choice below keeps the
other engines hidden underneath the 64 MiB/core of streaming traffic;
DMA floor = 64 MiB / 358 GB/s ~= 187 us, this config measures ~192-200 us):
  - x and the output stream as bf16 (host casts): halves HBM bytes for
    ~3e-3 relative error, well inside the 2e-2 budget. SE matmul weights and
    activations are bf16 too (HW fp32 matmul is ~4x slower).
  - the host pre-permutes x to [batch, block, partition, chan-blk, t] so each
    512-step block is one fully contiguous 2 MiB DMA per direction
    (256-col blocks measured much slower: per-block overheads dominate).
  - chunk-pooling runs as a pairwise bf16 tensor_add tree (DVE tensor_reduce
    only has a 1x uop; all-step-1 bf16 tensor_tensor gets the 2x packed mode).
  - the sigmoid reads the block's PSUM bank through a stride-0 AP and writes
    the gate expanded to ge=4 columns (gexp=5); the gating multiply runs as
    CHUNK/ge step-1 bf16 muls (2x mode) reusing that slice, so ACT does 4x
    less sigmoid work than full chunk expansion and gate tiles shrink
    16KB -> 4KB/partition. The per-channel-block bias is applied by a rank-1
    matmul on the otherwise idle PE.
  - the freed SBUF buys a deep software pipeline: the gate-multiply + store
    of block k are emitted pd=4 iterations later (pending queue), with
    xbufs=9 x-tile and gbufs=5 gate-tile rotation so loads run far ahead and
    the pool->scan->SE chain (~10 us) never stalls DVE. pd=1 measured 246 us,
    pd=3 217 us, this config ~192-200 us on the axon trn2 slope bench.
  - interleaving the two batch elements' blocks (b0k0, b1k0, b0k1, ...)
    doubles the slack on the EMA carry chain.
  - measured dead ends kept behind knobs: gpsimd tensor ops (Q7 software,
    slow), per-ob sigmoid (ACT instruction overhead), fine-grained tail
    blocks (tailf: extra per-block overheads exceed the drain savings),
    taper/rampup/storeeng (neutral), ps1b/ps2b deeper PSUM (worse).
"""

import contextlib

import ml_dtypes
import numpy as np

import concourse.bacc as bacc
import concourse.mybir as mybir
import concourse.tile as tile
from concourse import bass_utils

F32 = mybir.dt.float32
BF16 = mybir.dt.bfloat16
NP_BF16 = ml_dtypes.bfloat16

B = 16
C = 2048
L = 4096
CHUNK = 16
HID = 256
QMAX = 7
EPS = 1e-5
N_CORES = 8
BPC = B // N_CORES          # batches per core = 2
P = 128
NCB = C // P                # channel blocks = 16
NOC = HID // P              # hidden (SE bottleneck) blocks = 2
TBLK = 512                  # time elements per streamed block
NTB = L // TBLK             # time blocks = 8
TP = TBLK // CHUNK          # pooled steps per block = 32
CONTIG = 1                  # host pre-permutes x so block DMAs are contiguous
GEXP = 5                    # expand gate to 4 cols; 4 step-1 muls reuse it
TAILF = 0                   # split the last block per batch into TAILF fine
TFINE = 128                 # blocks of TFINE columns (measured slower; off)

_CACHE = {}


def _emit_body(nc, xd, outd, w1, w2, b1, b2, rmask, rlast,
               xpool, spool, carrypool, ps1, ps2, tblk=TBLK, ablate=(),
               contig=0, rdeng=0, muleng=0, treered=0, gexp=0,
               ones=None, b2row=None, interleave=0, fo=0, pd=1,
               gpool=None, storeeng=0, mulsplit=1, taper=0, rampup=0,
               xd2=None, outd2=None, tailf=0, tfine=128, fpool=None,
               trp=None):
    """One full pass over this core's two batch elements.

    Emission is software-pipelined: each (b, k) iteration emits this block's
    load/pool/scan/SE-gate, but the gate-multiply + store of a block pd
    iterations EARLIER. Engine instruction streams execute in order, so
    emitting mul(k) right after gate(k) would stall the whole DVE stream on
    the PE/ACT SE chain; delaying it pd blocks gives the SE chain ~pd
    iterations of slack before DVE blocks on the gate.
    """
    ntb = L // tblk
    tp = tblk // CHUNK
    if gpool is None:
        gpool = spool

    pending = []  # [(xt, gate, b, k), ...] awaiting mul+store, oldest first

    ge = {1: CHUNK, 2: CHUNK, 3: CHUNK, 4: 8, 5: 4}.get(gexp, CHUNK)

    def flush_one():
        xt, gate, tpb, oap = pending.pop(0)
        if "mul" not in ablate:
            x4 = xt[:, :, :tpb * CHUNK].rearrange(
                "p cb (tp ch) -> p cb tp ch", ch=CHUNK)
            if gexp:
                # gate expanded to ge columns per chunk: every mul is
                # step-1 bf16 on both operands -> DVE 2x packed mode;
                # CHUNK//ge passes reuse the same gate slice over the
                # chunk's sub-columns
                if muleng == 0:
                    step = (NCB + mulsplit - 1) // mulsplit
                    for s0 in range(0, NCB, step):
                        s1 = min(NCB, s0 + step)
                        for j in range(CHUNK // ge):
                            xs = x4[:, s0:s1, :, j * ge:(j + 1) * ge]
                            nc.vector.tensor_mul(xs, xs,
                                                 gate[:, s0:s1, :tpb])
                elif muleng == 1:
                    nc.gpsimd.tensor_mul(x4, x4, gate[:, :, :tpb])
                else:
                    h = NCB - NCB // 3
                    nc.vector.tensor_mul(x4[:, :h], x4[:, :h],
                                         gate[:, :h, :tpb])
                    nc.gpsimd.tensor_mul(x4[:, h:], x4[:, h:],
                                         gate[:, h:, :tpb])
            else:
                gb = gate[:, :, :tpb].unsqueeze(3).broadcast_to(
                    [P, NCB, tpb, CHUNK])
                if muleng == 0:
                    nc.vector.tensor_mul(x4, x4, gb)
                elif muleng == 1:
                    nc.gpsimd.tensor_mul(x4, x4, gb)
                else:
                    h = NCB // 2
                    nc.vector.tensor_mul(x4[:, :h], x4[:, :h], gb[:, :h])
                    nc.gpsimd.tensor_mul(x4[:, h:], x4[:, h:], gb[:, h:])
        seng = nc.gpsimd if storeeng else nc.scalar
        seng.dma_start(oap, xt[:, :, :tpb * CHUNK])

    def flush_pending(limit=0):
        while len(pending) > limit:
            flush_one()

    # Block list per batch: (ntb-1) full tblk blocks, then either one more
    # full block (tailf=0) or tailf fine blocks of tfine columns. Fine tail
    # blocks reuse the same full-size tiles via :tpb slices (no extra SBUF);
    # their much shorter pool->scan->SE->mul->store chain shrinks the
    # pipeline drain that trails the last load.
    def batch_blocks(b):
        blocks = []
        nmain = ntb - 1 if tailf else ntb
        for k in range(nmain):
            if contig:
                xap, oap = xd.ap()[b][k], outd.ap()[b][k]
            else:
                t0 = k * tblk
                xap = xd.ap()[b][:, :, t0:t0 + tblk].transpose([1, 0, 2])
                oap = outd.ap()[b][:, :, t0:t0 + tblk].transpose([1, 0, 2])
            blocks.append((b, tp, xap, oap))
        for f in range(tailf):
            blocks.append((b, tfine // CHUNK, xd2.ap()[b][f],
                           outd2.ap()[b][f]))
        return blocks

    per_b = [batch_blocks(b) for b in range(BPC)]
    nseq = len(per_b[0])
    if interleave:
        sched = [(s, *per_b[b][s]) for s in range(nseq) for b in range(BPC)]
    else:
        sched = [(s, *per_b[b][s]) for b in range(BPC) for s in range(nseq)]
    qcs = []
    for b in range(BPC):
        qc_t = carrypool.tile([P, NCB], F32, tag=f"qc{b}")
        qcs.append(qc_t)
    if True:
        for i, (seq, b, tpb, xap, oap) in enumerate(sched):
            lim = pd - 1
            if taper:
                # drain the backlog as the schedule ends so the final muls
                # overlap the last SE chains instead of queueing after them
                lim = min(lim, len(sched) - 1 - i)
            if rampup:
                # flush early blocks at shallow depth so the store stream
                # starts before the load run-ahead (xbufs) is exhausted and
                # the DMA queue never drains during the ramp
                lim = min(lim, max(i - 1, 0))
            qc = qcs[b]
            # fine tail blocks get a compact tile: DMA-ing into a :tpb slice
            # of the full tile would shatter the transfer into 256B-chunk
            # descriptors (below the 512B line-rate minimum)
            if tpb < tp:
                xt = fpool.tile([P, NCB, tpb * CHUNK], BF16, tag="xtf")
            else:
                xt = xpool.tile([P, NCB, tblk], BF16, tag="xt")
            nc.sync.dma_start(xt[:, :, :tpb * CHUNK], xap)
            x4 = xt[:, :, :tpb * CHUNK].rearrange(
                "p cb (tp ch) -> p cb tp ch", ch=CHUNK)

            sums = spool.tile([P, NCB, tp], F32, tag="sums")
            if "reduce" in ablate:
                nc.gpsimd.memset(sums[:], 0.01)
            elif treered:
                # pairwise-add tree: every level is all-bf16 step-1, so DVE
                # runs it in the 2x packed mode (tensor_reduce only has a 1x
                # uop and would cost ~2x more)
                tr = (trp or spool).tile([P, NCB, tp, 8], BF16, tag="tr")
                nc.vector.tensor_add(tr[:, :, :tpb], x4[:, :, :, 0:8],
                                     x4[:, :, :, 8:16])
                nc.vector.tensor_add(tr[:, :, :tpb, 0:4], tr[:, :, :tpb, 0:4],
                                     tr[:, :, :tpb, 4:8])
                nc.vector.tensor_add(tr[:, :, :tpb, 0:2], tr[:, :, :tpb, 0:2],
                                     tr[:, :, :tpb, 2:4])
                nc.vector.tensor_add(sums[:, :, :tpb], tr[:, :, :tpb, 0],
                                     tr[:, :, :tpb, 1])
            else:
                reng = nc.gpsimd if rdeng else nc.vector
                reng.reduce_sum(sums[:, :, :tpb], x4,
                                axis=mybir.AxisListType.X)

            if "se" in ablate:
                flush_pending(lim)
                pending.append((xt, sums, tpb, oap))
                continue
            if seq > 0:
                tmp = spool.tile([P, NCB], F32, tag="tmp")
                nc.vector.tensor_mul(tmp[:], qc[:], rlast[:])
                nc.vector.tensor_add(sums[:, :, 0], sums[:, :, 0], tmp[:])

            # scan runs at the tile's full width even for fine tail blocks
            # (stale columns >= tpb scan to garbage that is never read;
            # column j only depends on columns <= j, so q[:, :, :tpb] is
            # exact) -- keeps the scan AP dense/contiguous
            q = spool.tile([P, NCB, tp], BF16, tag="q")
            nc.vector.tensor_tensor_scan(
                q[:].rearrange("p cb tp -> p (cb tp)"),
                rmask[:].rearrange("p cb tp -> p (cb tp)"),
                sums[:].rearrange("p cb tp -> p (cb tp)"),
                initial=0.0,
                op0=mybir.AluOpType.mult,
                op1=mybir.AluOpType.add,
            )
            if seq < nseq - 1:
                nc.vector.tensor_copy(qc[:], q[:, :, tpb - 1])

            if not fo:
                flush_pending(lim)

            h1 = spool.tile([P, NOC, tp], BF16, tag="h1")
            for oc in range(NOC):
                acc = ps1.tile([P, tp], F32, tag="acc1")
                for cb in range(NCB):
                    nc.tensor.matmul(
                        acc[:, :tpb],
                        w1[:, cb, oc * P:(oc + 1) * P],
                        q[:, cb, :tpb],
                        start=(cb == 0),
                        stop=(cb == NCB - 1),
                    )
                nc.scalar.activation(
                    h1[:, oc, :tpb], acc[:, :tpb],
                    mybir.ActivationFunctionType.Relu,
                    bias=b1[:, oc:oc + 1],
                )

            if gexp == 3:
                # per-output-block sigmoid straight from PSUM with the bias
                # applied on ACT: no rank-1 bias matmuls on PE, and each
                # sigmoid only waits for its own ob's matmuls, so ACT
                # pipelines behind PE instead of waiting for the whole bank
                acc2 = ps2.tile([P, NCB, tp], F32, tag="acc2big")
                gate = gpool.tile([P, NCB, tp, CHUNK], BF16, tag="gate16")
                for ob in range(NCB):
                    for kc in range(NOC):
                        nc.tensor.matmul(
                            acc2[:, ob, :tpb],
                            w2[:, kc, ob * P:(ob + 1) * P],
                            h1[:, kc, :tpb],
                            start=(kc == 0),
                            stop=(kc == NOC - 1),
                        )
                    nc.scalar.activation(
                        gate[:, ob, :tpb],
                        acc2[:, ob, :tpb].unsqueeze(2).broadcast_to(
                            [P, tpb, CHUNK]),
                        mybir.ActivationFunctionType.Sigmoid,
                        bias=b2[:, ob:ob + 1],
                    )
            elif gexp:
                # all output blocks accumulate into one PSUM bank; per-block
                # bias lands via a 1-partition rank-1 matmul so a single
                # sigmoid (split in two for the PSUM 4K free-dim cap) can
                # write the gate already chunk-expanded for a 2x-mode mul
                acc2 = ps2.tile([P, NCB, tp], F32, tag="acc2big")
                for ob in range(NCB):
                    for kc in range(NOC):
                        nc.tensor.matmul(
                            acc2[:, ob, :tpb],
                            w2[:, kc, ob * P:(ob + 1) * P],
                            h1[:, kc, :tpb],
                            start=(kc == 0),
                            stop=False,
                        )
                    nc.tensor.matmul(
                        acc2[:, ob, :tpb],
                        b2row[0:1, ob * P:(ob + 1) * P],
                        ones[0:1, :tpb],
                        start=False,
                        stop=True,
                    )
                gate = gpool.tile([P, NCB, tp, ge], BF16, tag="gate16")
                if gexp == 2:
                    # sigmoid writes adjacent bf16 pairs; one int32-view copy
                    # replicates pairs to chunk width (half the elements)
                    g2 = spool.tile([P, NCB, tp, 2], BF16, tag="g2")
                    nc.scalar.activation(
                        g2[:, :, :tpb],
                        acc2[:, :, :tpb].unsqueeze(3).broadcast_to(
                            [P, NCB, tpb, 2]),
                        mybir.ActivationFunctionType.Sigmoid)
                    u32 = mybir.dt.uint32
                    nc.vector.tensor_copy(
                        gate[:, :, :tpb].bitcast(u32),
                        g2[:, :, :tpb].bitcast(u32).broadcast_to(
                            [P, NCB, tpb, CHUNK // 2]),
                    )
                else:
                    gb = acc2[:, :, :tpb].unsqueeze(3).broadcast_to(
                        [P, NCB, tpb, ge])
                    half = NCB // 2
                    nc.scalar.activation(
                        gate[:, :half, :tpb], gb[:, :half],
                        mybir.ActivationFunctionType.Sigmoid)
                    nc.scalar.activation(
                        gate[:, half:, :tpb], gb[:, half:],
                        mybir.ActivationFunctionType.Sigmoid)
            else:
                gate = gpool.tile([P, NCB, tp], BF16, tag="gate")
                for ob in range(NCB):
                    acc2 = ps2.tile([P, tp], F32, tag="acc2")
                    for kc in range(NOC):
                        nc.tensor.matmul(
                            acc2[:, :tpb],
                            w2[:, kc, ob * P:(ob + 1) * P],
                            h1[:, kc, :tpb],
                            start=(kc == 0),
                            stop=(kc == NOC - 1),
                        )
                    nc.scalar.activation(
                        gate[:, ob, :tpb], acc2[:, :tpb],
                        mybir.ActivationFunctionType.Sigmoid,
                        bias=b2[:, ob:ob + 1],
                    )

            if fo:
                flush_pending(lim)
            pending.append((xt, gate, tpb, oap))
    flush_pending()


def _build_module(repeat=1, tblk=TBLK, xbufs=9, sbufs=2, ps1b=2, ps2b=4, ablate=(),
                  contig=CONTIG, rdeng=0, muleng=0, treered=1, gexp=GEXP,
                  interleave=1, fo=0, pd=4, gbufs=None, storeeng=0,
                  mulsplit=1, taper=0, rampup=0, tailf=0, tfine=TFINE,
                  fbufs=2, trbufs=1):
    """Build the per-core module. repeat>1 wraps the body in a hardware loop
    that re-runs it (idempotently) for slope-based device timing."""
    tp = tblk // CHUNK
    ntb = L // tblk
    nc = bacc.Bacc("TRN2", target_bir_lowering=False, debug=False,
                   num_devices=N_CORES)

    if tailf:
        assert contig and tailf * tfine == tblk
        xshape = [BPC, ntb - 1, P, NCB, tblk]
        tshape = [BPC, tailf, P, NCB, tfine]
        xd2 = nc.dram_tensor("x2", tshape, BF16, kind="ExternalInput")
        outd2 = nc.dram_tensor("outt", tshape, BF16, kind="ExternalOutput")
    else:
        xshape = [BPC, ntb, P, NCB, tblk] if contig else [BPC, NCB, P, L]
        xd2 = outd2 = None
    xd = nc.dram_tensor("x", xshape, BF16, kind="ExternalInput")
    w1d = nc.dram_tensor("w1t", [P, NCB, HID], BF16, kind="ExternalInput")
    w2d = nc.dram_tensor("w2t", [P, NOC, C], BF16, kind="ExternalInput")
    b1d = nc.dram_tensor("b1t", [P, NOC], F32, kind="ExternalInput")
    b2d = nc.dram_tensor("b2t", [P, NCB], F32, kind="ExternalInput")
    rmd = nc.dram_tensor("rmask", [P, NCB, tp], F32, kind="ExternalInput")
    rld = nc.dram_tensor("rlast", [P, NCB], F32, kind="ExternalInput")
    b2rd = (nc.dram_tensor("b2r", [1, C], BF16, kind="ExternalInput")
            if gexp else None)
    outd = nc.dram_tensor("out", xshape, BF16, kind="ExternalOutput")

    if gbufs is None:
        gbufs = max(sbufs, pd + 1)
    with tile.TileContext(nc) as tc:
        with (
            tc.tile_pool(name="const", bufs=1) as cpool,
            tc.tile_pool(name="xp", bufs=xbufs) as xpool,
            tc.tile_pool(name="small", bufs=sbufs) as spool,
            tc.tile_pool(name="gp", bufs=gbufs) as gpool,
            tc.tile_pool(name="trp", bufs=trbufs) as trp,
            tc.tile_pool(name="fp", bufs=fbufs) as fpool,
            tc.tile_pool(name="carry", bufs=1) as carrypool,
            tc.tile_pool(name="ps1", bufs=ps1b, space="PSUM") as ps1,
            tc.tile_pool(name="ps2", bufs=ps2b, space="PSUM") as ps2,
        ):
            w1 = cpool.tile([P, NCB, HID], BF16)
            w2 = cpool.tile([P, NOC, C], BF16)
            b1 = cpool.tile([P, NOC], F32)
            b2 = cpool.tile([P, NCB], F32)
            rmask = cpool.tile([P, NCB, tp], F32)
            rlast = cpool.tile([P, NCB], F32)
            nc.gpsimd.dma_start(w1[:], w1d.ap())
            nc.gpsimd.dma_start(w2[:], w2d.ap())
            nc.gpsimd.dma_start(b1[:], b1d.ap())
            nc.gpsimd.dma_start(b2[:], b2d.ap())
            nc.gpsimd.dma_start(rmask[:], rmd.ap())
            nc.gpsimd.dma_start(rlast[:], rld.ap())
            if gexp:
                b2row = cpool.tile([1, C], BF16)
                nc.gpsimd.dma_start(b2row[:], b2rd.ap())
                ones = cpool.tile([1, tp], BF16)
                nc.gpsimd.memset(ones[:], 1.0)
            else:
                b2row = ones = None

            rep = tc.For_i(0, repeat, 1) if repeat > 1 else contextlib.nullcontext()
            with rep:
                _emit_body(nc, xd, outd, w1, w2, b1, b2, rmask, rlast,
                           xpool, spool, carrypool, ps1, ps2, tblk=tblk,
                           ablate=ablate, contig=contig, rdeng=rdeng,
                           muleng=muleng, treered=treered, gexp=gexp,
                           ones=ones, b2row=b2row, interleave=interleave,
                           fo=fo, pd=pd, gpool=gpool, storeeng=storeeng,
                           mulsplit=mulsplit, taper=taper, rampup=rampup,
                           xd2=xd2, outd2=outd2, tailf=tailf, tfine=tfine,
                           fpool=fpool, trp=trp)

    nc.compile()
    return nc


def _fake_quant(w):
    w = np.asarray(w, np.float32)
    scale = (np.max(np.abs(w), axis=1, keepdims=True).astype(np.float32)
             / np.float32(QMAX) + np.float32(EPS)).astype(np.float32)
    wq = np.clip(np.round(w / scale), -QMAX, QMAX).astype(np.float32) * scale
    return wq.astype(np.float32)


def _host_prep(w1, b1, w2, b2, ema_r, tp=TP):
    r = np.asarray(ema_r, np.float32)
    s = ((np.float32(1.0) - r) / np.float32(CHUNK)).astype(np.float32)

    w1s = (_fake_quant(w1) * s[None, :]).astype(np.float32)        # [HID, C]
    w1t = np.ascontiguousarray(
        w1s.T.reshape(NCB, P, HID).transpose(1, 0, 2)).astype(NP_BF16)
    w2q = _fake_quant(w2)                                          # [C, HID]
    w2t = np.ascontiguousarray(
        w2q.T.reshape(NOC, P, C).transpose(1, 0, 2)).astype(NP_BF16)
    b1t = np.ascontiguousarray(np.asarray(b1, np.float32).reshape(NOC, P).T)
    b2t = np.ascontiguousarray(np.asarray(b2, np.float32).reshape(NCB, P).T)

    rpb = r.reshape(NCB, P).T                                      # [P, NCB]
    rmask = np.repeat(rpb[:, :, None], tp, axis=2).astype(np.float32)
    rmask[:, :, 0] = 0.0
    rlast = np.ascontiguousarray(rpb)
    return w1t, w2t, b1t, b2t, np.ascontiguousarray(rmask), rlast


def _make_in_maps(x, w1, b1, w2, b2, ema_r, tp=TP, tailf=TAILF, tfine=TFINE):
    w1t, w2t, b1t, b2t, rmask, rlast = _host_prep(w1, b1, w2, b2, ema_r, tp=tp)
    xh = np.asarray(x, np.float32).reshape(B, NCB, P, L).astype(NP_BF16)
    x2 = None
    if CONTIG:
        tblk = tp * CHUNK
        xh = np.ascontiguousarray(
            xh.reshape(B, NCB, P, L // tblk, tblk).transpose(0, 3, 2, 1, 4))
        if tailf:
            # last block per batch, re-sliced into tailf fine blocks
            x2 = np.ascontiguousarray(
                xh[:, -1].reshape(B, P, NCB, tailf, tfine)
                .transpose(0, 3, 1, 2, 4))
            xh = np.ascontiguousarray(xh[:, :-1])
    m = {"w1t": w1t, "w2t": w2t, "b1t": b1t, "b2t": b2t,
         "rmask": rmask, "rlast": rlast}
    if GEXP:
        m["b2r"] = np.asarray(b2, np.float32).reshape(1, C).astype(NP_BF16)
    maps = []
    for c in range(N_CORES):
        mc = dict(m, x=xh[c * BPC:(c + 1) * BPC])
        if x2 is not None:
            mc["x2"] = x2[c * BPC:(c + 1) * BPC]
        maps.append(mc)
    return maps


def _assemble(out_main, out_tail, tblk=TBLK, tailf=TAILF, tfine=TFINE):
    """Reassemble the full [B, C, L] f32 output from the per-core module
    outputs (concatenated over cores along axis 0 = batch)."""
    ntb = L // tblk
    if tailf:
        full = np.empty((B, NCB, P, L), np.float32)
        om = np.asarray(out_main, np.float32)
        full[:, :, :, :L - tblk] = om.transpose(0, 3, 2, 1, 4).reshape(
            B, NCB, P, L - tblk)
        ot = np.asarray(out_tail, np.float32)
        full[:, :, :, L - tblk:] = ot.transpose(0, 3, 2, 1, 4).reshape(
            B, NCB, P, tblk)
    else:
        om = np.asarray(out_main, np.float32).reshape(B, ntb, P, NCB, tblk)
        full = np.ascontiguousarray(om.transpose(0, 3, 2, 1, 4))
    return full.reshape(B, C, L)


def kernel(x, w1, b1, w2, b2, ema_r):
    if "nc" not in _CACHE:
        _CACHE["nc"] = _build_module()
    nc = _CACHE["nc"]

    in_maps = _make_in_maps(x, w1, b1, w2, b2, ema_r)
    res = bass_utils.run_bass_kernel_spmd(nc, in_maps,
                                          core_ids=list(range(N_CORES)))
    if CONTIG:
        nmain = NTB - 1 if TAILF else NTB
        out = np.empty((B, nmain, P, NCB, TBLK), np.float32)
        outt = (np.empty((B, TAILF, P, NCB, TFINE), np.float32)
                if TAILF else None)
        for c in range(N_CORES):
            out[c * BPC:(c + 1) * BPC] = np.asarray(res.results[c]["out"])
            if TAILF:
                outt[c * BPC:(c + 1) * BPC] = np.asarray(
                    res.results[c]["outt"])
        return _assemble(out, outt)
    out = np.empty((B, NCB, P, L), np.float32)
    for c in range(N_CORES):
        out[c * BPC:(c + 1) * BPC] = np.asarray(res.results[c]["out"])
    return out.reshape(B, C, L)

